# revision 18
# baseline (speedup 1.0000x reference)
"""Single-head attention (B=4, N=2048, D=1024), scores scaled by 10.

Sharding: 8 cores = (batch, query-half), fully collective-free. Core
2b+h owns queries [1024h:1024(h+1)] of batch b and computes K^T for
ALL 2048 keys locally (the host feeds the global x halves, which are
identical for both cores of a pair, so the SPMD program is symmetric).

The value path uses associativity: instead of V = x@wv then P@V
(4.3e9 MACs/core), compute Y = P@x_global then O = Y@wv^T
(3.2e9 MACs/core) - the contraction middle dim collapses from 2048
keys to this core's 1024 queries, and wv never touches x at all.

Numerics: single-pass fp16 everywhere (fp32 PSUM accumulation),
rel err ~6e-3 against the fp32 reference (gate 2e-2).

Schedule: the attention loop is software-pipelined. While QK(c) runs
on the tensor engine, chunk c-1's softmax runs beside it (flat DVE max
tree, rank-1 max/recip broadcasts as tiny matmuls slotted between QK
accumulation groups, exp on the scalar engine in kt-pairs), then the
PE rolls straight into sums/Y/O of c-1. DMA dispatches (~1us each,
serialized per issuing engine) are spread across the sync, scalar and
gpsimd queues, with latency-critical tiles split by partition half for
ring parallelism.
"""

import numpy as np

B, SEQ, D = 4, 2048, 1024
NQ = 1024          # queries per core
QCH = 256          # attention q-chunk
NCH = NQ // QCH
NCORES = 8
DT = D // 128      # 8 d-tiles
ET = D // 128      # 8 e-tiles
KT = SEQ // 128    # 16 k-tiles

_BUILT = {}


def _build():
    if "nc" in _BUILT:
        return _BUILT["nc"]
    from contextlib import ExitStack

    import concourse.bass as bass  # noqa: F401
    import concourse.mybir as mybir
    import concourse.tile as tile
    from concourse import bacc

    dt = mybir.dt
    F32, F16 = dt.float32, dt.float16
    AL = mybir.AluOpType
    EXP = mybir.ActivationFunctionType.Exp

    nc = bacc.Bacc("TRN2", target_bir_lowering=False, debug=False)

    # xp: own-half x^T [d, n];  xa/xb: global x^T halves [d, k];
    # xn: global x (untransposed) [k, d] for the Y contraction;
    # weights packed so each e-tile DMA is contiguous per partition
    xp_d = nc.dram_tensor("xp", [128, DT * NQ], F16, kind="ExternalInput")
    xa_d = nc.dram_tensor("xa", [128, DT * NQ], F16, kind="ExternalInput")
    xb_d = nc.dram_tensor("xb", [128, DT * NQ], F16, kind="ExternalInput")
    xn_d = nc.dram_tensor("xn", [128, KT * D], F16, kind="ExternalInput")
    wq_d = nc.dram_tensor("wq", [128, ET * DT * 128], F16, kind="ExternalInput")
    wk_d = nc.dram_tensor("wk", [128, ET * DT * 128], F16, kind="ExternalInput")
    wv_d = nc.dram_tensor("wv", [128, DT * D], F16, kind="ExternalInput")
    ot_d = nc.dram_tensor("ot", [128, ET * NQ], F16, kind="ExternalOutput")

    xp_r = xp_d.ap().rearrange("p (t n) -> p t n", t=DT)
    xa_r = xa_d.ap().rearrange("p (t n) -> p t n", t=DT)
    xb_r = xb_d.ap().rearrange("p (t n) -> p t n", t=DT)
    xn_r = xn_d.ap().rearrange("p (t e) -> p t e", t=KT)
    wq_r = wq_d.ap().rearrange("p (e tc) -> p e tc", e=ET)
    wk_r = wk_d.ap().rearrange("p (e tc) -> p e tc", e=ET)
    wv_r = wv_d.ap().rearrange("p (t e) -> p t e", t=DT)
    ot_r = ot_d.ap().rearrange("p (t q) -> p t q", t=ET)

    with tile.TileContext(nc) as tc, ExitStack() as ctx:
        qk_pool = ctx.enter_context(tc.tile_pool(name="qk", bufs=1))
        qt = qk_pool.tile([128, ET, NQ], F16, tag="qt")
        ktt = qk_pool.tile([128, ET, SEQ], F16, tag="ktt")
        xnf = qk_pool.tile([128, KT, D], F16, tag="xnf")
        wvA = qk_pool.tile([128, DT, D], F16, tag="wvA")

        const_pool = ctx.enter_context(tc.tile_pool(name="const", bufs=1))
        ten32 = const_pool.tile([1, 128], F32, tag="ten32")
        one32 = const_pool.tile([1, 128], F32, tag="one32")
        ones16 = const_pool.tile([128, 1], F16, tag="ones16")
        nc.vector.memset(ten32[:], 10.0)
        nc.vector.memset(one32[:], 1.0)
        nc.vector.memset(ones16[:], 1.0)

        # ---------------- Projections (all single-pass fp16) --------------
        with (
            tc.tile_pool(name="xspan", bufs=1) as xspan,
            tc.tile_pool(name="wall", bufs=1) as wall,
            tc.tile_pool(name="psA", bufs=4, space="PSUM") as psA,
        ):
            x_t = xspan.tile([128, DT, NQ], F16, tag="x")
            xg = xspan.tile([128, DT, SEQ], F16, tag="xg")
            wkF = wall.tile([128, ET, DT * 128], F16, tag="wkF")
            wqF = wall.tile([128, ET, DT * 128], F16, tag="wqF")
            # loads spread across the three DMA-capable queues; first-needed
            # tiles split by partition half for ring parallelism
            nc.scalar.dma_start(wkF[0:64, 0, :], wk_r[0:64, 0, :])
            nc.scalar.dma_start(wkF[64:128, 0, :], wk_r[64:128, 0, :])
            for dti in range(DT):
                nc.sync.dma_start(xg[0:64, dti, 0:NQ], xa_r[0:64, dti, :])
                nc.sync.dma_start(xg[64:128, dti, 0:NQ], xa_r[64:128, dti, :])
                nc.scalar.dma_start(xg[0:64, dti, NQ:SEQ], xb_r[0:64, dti, :])
                nc.scalar.dma_start(xg[64:128, dti, NQ:SEQ], xb_r[64:128, dti, :])
            for et in range(1, ET):
                nc.scalar.dma_start(wkF[0:64, et, :], wk_r[0:64, et, :])
                nc.scalar.dma_start(wkF[64:128, et, :], wk_r[64:128, et, :])
            for dti in range(DT):
                nc.gpsimd.dma_start(x_t[:, dti, :], xp_r[:, dti, :])
            for kt in range(KT):
                nc.gpsimd.dma_start(xnf[:, kt, :], xn_r[:, kt, :])
            for ec in range(2):
                nc.sync.dma_start(
                    wvA[:, :, 512 * ec : 512 * (ec + 1)],
                    wv_r[:, :, 512 * ec : 512 * (ec + 1)],
                )
            for et in range(ET):
                nc.scalar.dma_start(wqF[:, et, :], wq_r[:, et, :])

            # ---- Phase K: FULL K^T projection, evac straight to SBUF -----
            for et in range(ET):
                ps0 = psA.tile([128, 512], F32, tag="psA")
                ps1 = psA.tile([128, 512], F32, tag="psA")
                ps2 = psA.tile([128, 512], F32, tag="psA")
                ps3 = psA.tile([128, 512], F32, tag="psA")
                ps = (ps0, ps1, ps2, ps3)
                # dti outer so the first et paces with the incoming xg stream
                for dti in range(DT):
                    for chn in range(4):
                        nc.tensor.matmul(
                            ps[chn][:],
                            wkF[:, et, 128 * dti : 128 * (dti + 1)],
                            xg[:, dti, 512 * chn : 512 * (chn + 1)],
                            start=(dti == 0),
                            stop=(dti == DT - 1),
                        )
                for chn in range(4):
                    nc.vector.tensor_copy(
                        ktt[:, et, 512 * chn : 512 * (chn + 1)], ps[chn][:]
                    )

            # ---- Phase Q: own-half Q^T projection ------------------------
            for et in range(ET):
                for chn in range(2):
                    n0 = 512 * chn
                    ps = psA.tile([128, 512], F32, tag="psA")
                    for dti in range(DT):
                        nc.tensor.matmul(
                            ps[:],
                            wqF[:, et, 128 * dti : 128 * (dti + 1)],
                            x_t[:, dti, n0 : n0 + 512],
                            start=(dti == 0),
                            stop=(dti == DT - 1),
                        )
                    nc.vector.tensor_copy(qt[:, et, n0 : n0 + 512], ps[:])

        # ---------------- Attention, q-chunked, software-pipelined --------
        with (
            tc.tile_pool(name="stp", bufs=2) as stpool,
            tc.tile_pool(name="pp", bufs=2) as ppool,
            tc.tile_pool(name="yy", bufs=2) as ypool,
            tc.tile_pool(name="tree", bufs=2) as treepool,
            tc.tile_pool(name="aux", bufs=2) as auxpool,
            tc.tile_pool(name="osb", bufs=3) as outpool,
            tc.tile_pool(name="psS", bufs=3, space="PSUM") as psS,
            tc.tile_pool(name="psW", bufs=3, space="PSUM") as psW,
            tc.tile_pool(name="psX", bufs=1, space="PSUM") as psX,
            tc.tile_pool(name="psR", bufs=1, space="PSUM") as psR,
        ):
            sts = [None] * NCH   # scores [128, KT, QCH] f32
            pts = [None] * NCH   # exp(10(s-max)) [128, KT, QCH] f16
            yts = [None] * NCH   # Y = P @ x_global [128(d), DT, QCH] f16
            m1s = [None] * NCH   # per-query max row, doubled [1, 2*QCH]
            mbs = [None] * NCH   # broadcast 10*max, doubled [128, 2*QCH]

            def tree_fold_max(c, t8pre=None):
                # rowwise max over kt as a flat tree, then partition-reduce
                # via 32-partition folds + DVE 32x32 block transposes
                st = sts[c]
                if t8pre is None:
                    t8 = treepool.tile([128, 8, QCH], F32, tag="t8", name="t8")
                    nc.vector.tensor_max(
                        t8[:].rearrange("p a q -> p (a q)"),
                        st[:, 0:8, :].rearrange("p a q -> p (a q)"),
                        st[:, 8:16, :].rearrange("p a q -> p (a q)"),
                    )
                    nc.vector.tensor_max(
                        t8[:, 0:4, :].rearrange("p a q -> p (a q)"),
                        t8[:, 0:4, :].rearrange("p a q -> p (a q)"),
                        t8[:, 4:8, :].rearrange("p a q -> p (a q)"),
                    )
                else:
                    # t8pre[:,0:4] already = max(st[0:4], st[4:8])
                    t8 = t8pre
                    nc.vector.tensor_max(
                        t8[:, 4:8, :].rearrange("p a q -> p (a q)"),
                        st[:, 8:12, :].rearrange("p a q -> p (a q)"),
                        st[:, 12:16, :].rearrange("p a q -> p (a q)"),
                    )
                    nc.vector.tensor_max(
                        t8[:, 0:4, :].rearrange("p a q -> p (a q)"),
                        t8[:, 0:4, :].rearrange("p a q -> p (a q)"),
                        t8[:, 4:8, :].rearrange("p a q -> p (a q)"),
                    )
                nc.vector.tensor_max(
                    t8[:, 0:2, :].rearrange("p a q -> p (a q)"),
                    t8[:, 0:2, :].rearrange("p a q -> p (a q)"),
                    t8[:, 2:4, :].rearrange("p a q -> p (a q)"),
                )
                nc.vector.tensor_max(t8[:, 0, :], t8[:, 0, :], t8[:, 1, :])
                fold4 = treepool.tile([32, 4, QCH], F32, tag="fold4", name="f4")
                for a in range(4):
                    nc.sync.dma_start(
                        fold4[:, a, :], t8[32 * a : 32 * (a + 1), 0, :]
                    )
                nc.vector.tensor_max(fold4[:, 0, :], fold4[:, 0, :], fold4[:, 1, :])
                nc.vector.tensor_max(fold4[:, 2, :], fold4[:, 2, :], fold4[:, 3, :])
                nc.vector.tensor_max(fold4[:, 0, :], fold4[:, 0, :], fold4[:, 2, :])
                t32t = treepool.tile([32, QCH], F32, tag="t32t", name="t32t")
                nc.vector.transpose(t32t[:], fold4[:, 0, :])
                mx32 = treepool.tile([32, 32], F32, tag="mx32", name="mx32")
                nc.vector.memset(mx32[:], 0.0)
                nc.vector.reduce_max(
                    mx32[:, 0 : QCH // 32],
                    t32t[:].rearrange("p (j c) -> p j c", c=32),
                    axis=mybir.AxisListType.X,
                )
                mx32t = treepool.tile([32, 32], F32, tag="mx32t", name="mx32t")
                nc.vector.transpose(mx32t[:], mx32[:])
                # doubled row so pairwise [128, 2*QCH] ops need no broadcast
                m1row = treepool.tile([1, 2 * QCH], F32, tag="m1row", name="m1row")
                nc.sync.dma_start(m1row[0:1, 0:QCH], mx32t[0 : QCH // 32, :])
                nc.sync.dma_start(m1row[0:1, QCH : 2 * QCH], mx32t[0 : QCH // 32, :])
                m1s[c] = m1row

            def head(c, maxb_ps):
                maxb = auxpool.tile([128, 2 * QCH], F32, tag="maxb", name="maxb")
                mbs[c] = maxb
                nc.vector.tensor_copy(maxb[:], maxb_ps[:])
                p_t = ppool.tile([128, KT, QCH], F16, tag="p", name="p_t")
                pts[c] = p_t

            def head_pair(c, j):
                # shift+exp of chunk c, kt-pair j
                st, p_t = sts[c], pts[c]
                sp = st[:, 2 * j : 2 * j + 2, :].rearrange("p a q -> p (a q)")
                nc.vector.scalar_tensor_tensor(
                    sp, sp, 10.0, mbs[c][:], op0=AL.mult, op1=AL.subtract
                )
                nc.scalar.activation(
                    p_t[:, 2 * j : 2 * j + 2, :].rearrange("p a q -> p (a q)"),
                    sp,
                    EXP,
                )

            def qk_block(c, prev, hoist=False):
                # QK of chunk c in kt-pairs sharing a PSUM bank; chunk prev's
                # max-broadcast + shift + exp interleave into the streams
                q0 = QCH * c
                st = stpool.tile([128, KT, QCH], F32, tag="st", name="st")
                sts[c] = st
                t8h = None
                for j in range(KT // 2):
                    ps = psS.tile([128, 2 * QCH], F32, tag="psS", name="ps")
                    for half in range(2):
                        kt = 2 * j + half
                        k0 = 128 * kt
                        for et in range(ET):
                            nc.tensor.matmul(
                                ps[:, QCH * half : QCH * (half + 1)],
                                ktt[:, et, k0 : k0 + 128],
                                qt[:, et, q0 : q0 + QCH],
                                start=(et == 0),
                                stop=(et == ET - 1),
                            )
                    if prev is not None and j == 1:
                        maxb_ps = psX.tile(
                            [128, 2 * QCH], F32, tag="bc", name="mb"
                        )
                        nc.tensor.matmul(
                            maxb_ps[:], ten32[:], m1s[prev][:],
                            start=True, stop=True,
                        )
                    nc.vector.tensor_copy(
                        st[:, 2 * j : 2 * j + 2, :].rearrange("p a q -> p (a q)"),
                        ps[:],
                    )
                    if hoist and j == 3:
                        # first tree level over kt 0..7 while QK continues
                        t8h = treepool.tile([128, 8, QCH], F32, tag="t8", name="t8h")
                        nc.vector.tensor_max(
                            t8h[:, 0:4, :].rearrange("p a q -> p (a q)"),
                            st[:, 0:4, :].rearrange("p a q -> p (a q)"),
                            st[:, 4:8, :].rearrange("p a q -> p (a q)"),
                        )
                    if prev is not None:
                        if j == 1:
                            head(prev, maxb_ps)
                        if j >= 2:
                            head_pair(prev, j - 2)
                            if j == KT // 2 - 1:
                                head_pair(prev, j - 1)
                                head_pair(prev, j)
                return t8h

            def sums_part(c):
                # key-sums of exp as rank-1 ones matmuls + recip + broadcast
                p_t = pts[c]
                sum_ps = psR.tile([1, QCH], F32, tag="sum", name="sum_ps")
                for kt in range(KT):
                    nc.tensor.matmul(
                        sum_ps[:],
                        ones16[:],
                        p_t[:, kt, :],
                        start=(kt == 0),
                        stop=(kt == KT - 1),
                    )
                recrow = treepool.tile([1, 2 * QCH], F32, tag="recrow", name="rr")
                nc.vector.reciprocal(recrow[0:1, 0:QCH], sum_ps[:])
                nc.vector.reciprocal(recrow[0:1, QCH : 2 * QCH], sum_ps[:])
                recb_ps = psX.tile([128, 2 * QCH], F32, tag="bc", name="rb")
                nc.tensor.matmul(
                    recb_ps[:], one32[:], recrow[:], start=True, stop=True
                )
                return recb_ps

            def y_part(c):
                # Y = P @ x_global: contract all 2048 keys down to this
                # core's queries; dt-pairs share a PSUM bank
                p_t = pts[c]
                yt = ypool.tile([128, DT, QCH], F16, tag="yt", name="yt")
                yts[c] = yt
                for dj in range(DT // 2):
                    ps = psW.tile([128, 2 * QCH], F32, tag="psW", name="psy")
                    for half in range(2):
                        d0 = 128 * (2 * dj + half)
                        for kt in range(KT):
                            nc.tensor.matmul(
                                ps[:, QCH * half : QCH * (half + 1)],
                                xnf[:, kt, d0 : d0 + 128],
                                p_t[:, kt, :],
                                start=(kt == 0),
                                stop=(kt == KT - 1),
                            )
                    nc.vector.tensor_copy(
                        yt[:, 2 * dj : 2 * dj + 2, :].rearrange("p a q -> p (a q)"),
                        ps[:],
                    )

            def o_part(c, recb_ps, hoist_c=None):
                # O^T = wv^T @ Y, scaled by 1/sum; et-pairs share a bank.
                # when hoist_c is set, that chunk's max-broadcast + shift +
                # exp interleave here so its exp is ready early
                q0 = QCH * c
                yt = yts[c]
                recb = auxpool.tile([128, 2 * QCH], F32, tag="recb", name="recb")
                nc.vector.tensor_copy(recb[:], recb_ps[:])
                for ej in range(ET // 2):
                    ops = psW.tile([128, 2 * QCH], F32, tag="psW", name="pso")
                    for half in range(2):
                        e0 = 128 * (2 * ej + half)
                        for dti in range(DT):
                            nc.tensor.matmul(
                                ops[:, QCH * half : QCH * (half + 1)],
                                wvA[:, dti, e0 : e0 + 128],
                                yt[:, dti, :],
                                start=(dti == 0),
                                stop=(dti == DT - 1),
                            )
                    if hoist_c is not None:
                        if ej == 0:
                            maxb_ps = psX.tile(
                                [128, 2 * QCH], F32, tag="bc", name="mb2"
                            )
                            nc.tensor.matmul(
                                maxb_ps[:], ten32[:], m1s[hoist_c][:],
                                start=True, stop=True,
                            )
                            head(hoist_c, maxb_ps)
                        elif ej == 1:
                            for j in range(4):
                                head_pair(hoist_c, j)
                        elif ej == 2:
                            for j in range(4, 8):
                                head_pair(hoist_c, j)
                    osb = outpool.tile([128, 2 * QCH], F16, tag="osb", name="osb")
                    nc.vector.scalar_tensor_tensor(
                        osb[:], ops[:], 1.0, recb[:], op0=AL.mult, op1=AL.mult
                    )
                    nc.sync.dma_start(
                        ot_r[:, 2 * ej : 2 * ej + 2, q0 : q0 + QCH],
                        osb[:].rearrange("p (a q) -> p a q", a=2),
                    )

            # software pipeline: tree(c) is emitted between chunk c-1's sums
            # and Y so m1row(c) is ready before QK(c+1)'s broadcast; the last
            # chunk's softmax interleaves into o_part(c-2)
            qk_block(0, None)
            tree_fold_max(0)
            for c in range(1, NCH - 1):
                qk_block(c, c - 1)
                rb = sums_part(c - 1)
                tree_fold_max(c)
                y_part(c - 1)
                o_part(c - 1, rb)
            t8h = qk_block(NCH - 1, NCH - 2, hoist=True)
            rb = sums_part(NCH - 2)
            tree_fold_max(NCH - 1, t8h)
            y_part(NCH - 2)
            o_part(NCH - 2, rb, hoist_c=NCH - 1)
            rb = sums_part(NCH - 1)
            y_part(NCH - 1)
            o_part(NCH - 1, rb)

    nc.compile()
    _BUILT["nc"] = nc
    return nc


def _prep_inputs(x, q_w, k_w, v_w):
    f16 = np.float16

    def pack_w_lhsT(w):
        # w is [out=e, in=d]; pack [p, eb, t, c] = w[eb*128+c, t*128+p]
        a = w.T.astype(f16).reshape(DT, 128, ET, 128)
        return np.ascontiguousarray(a.transpose(1, 2, 0, 3)).reshape(
            128, ET * DT * 128
        )

    def pack_w_rhs(w):
        # pack [p, t, e] = w.T[t*128+p, e]
        a = w.T.astype(f16).reshape(DT, 128, D)
        return np.ascontiguousarray(a.transpose(1, 0, 2)).reshape(128, DT * D)

    wq = pack_w_lhsT(q_w)
    wk = pack_w_lhsT(k_w)
    wv = pack_w_rhs(v_w)

    def pack_x_t(xslab):
        # x^T [d, n] packed [p, t, n] = x^T[t*128+p, n]
        xt = np.asarray(xslab).T.astype(f16)
        return np.ascontiguousarray(
            xt.reshape(DT, 128, NQ).transpose(1, 0, 2)
        ).reshape(128, DT * NQ)

    in_maps = []
    for b in range(B):
        xh = [pack_x_t(x[b, NQ * g : NQ * (g + 1)]) for g in range(2)]
        # xn: x global [k, d] packed [p, kt, d] = x[kt*128+p, d]
        xnb = np.asarray(x[b]).astype(f16)
        xn = np.ascontiguousarray(
            xnb.reshape(KT, 128, D).transpose(1, 0, 2)
        ).reshape(128, KT * D)
        for h in range(2):
            in_maps.append({
                "xp": xh[h], "xa": xh[0], "xb": xh[1], "xn": xn,
                "wq": wq, "wk": wk, "wv": wv,
            })
    return in_maps


def run(x, q_w, k_w, v_w, trace=False):
    from concourse.bass_utils import run_bass_kernel_spmd

    nc = _build()
    in_maps = _prep_inputs(x, q_w, k_w, v_w)
    res = run_bass_kernel_spmd(nc, in_maps, list(range(NCORES)), trace=trace)
    out = np.empty((B, SEQ, D), np.float32)
    for core in range(NCORES):
        b, h = divmod(core, 2)
        ot = res.results[core]["ot"].astype(np.float32).reshape(128, ET, NQ)
        out[b, NQ * h : NQ * (h + 1)] = (
            ot.transpose(1, 0, 2).reshape(D, NQ).T
        )
    return out, res


def kernel(x, q_w, k_w, v_w):
    x = np.asarray(x, np.float32)
    q_w = np.asarray(q_w, np.float32)
    k_w = np.asarray(k_w, np.float32)
    v_w = np.asarray(v_w, np.float32)
    out, _ = run(x, q_w, k_w, v_w, trace=False)
    return out


if __name__ == "__main__":
    rng = np.random.default_rng(0)
    x = rng.standard_normal((B, SEQ, D), np.float32)
    s = 1.0 / np.sqrt(D)
    q_w = rng.uniform(-s, s, (D, D)).astype(np.float32)
    k_w = rng.uniform(-s, s, (D, D)).astype(np.float32)
    v_w = rng.uniform(-s, s, (D, D)).astype(np.float32)
    out = kernel(x, q_w, k_w, v_w)
    print(out.shape, out.dtype)
